# revision 15
# baseline (speedup 1.0000x reference)
"""EMAVectorQuantizer forward on 8 Trainium2 NeuronCores (Bass/Tile).

Reference computation:
    d[n,k] = ||z_n||^2 + ||w_k||^2 - 2 z_n.w_k          n<65536, k<1024, D=256
    idx[n] = argmin_k d[n,k]   (first occurrence)
    z_q    = w[idx];  loss = 0.25*mean((z_q - z)^2);  z_q_st = z + (z_q - z)

Strategy (data parallel over tokens, 8 cores x 8192 tokens):
  argmin_k d = argmax_k q,  q[n,k] = 2 z_n.w_k - ||w_k||^2  (||z||^2 drops).

  Matmul precision: the PE's fast fp32 mode (float32r) rounds both operands
  to ~10 mantissa bits (TF32-like) but multiplies those exactly.  We split
  each operand at 10 mantissa bits (x = xh + xl, xh = trunc10(x)) and
  compute q = zh.wh + zh.wl + zl.wh (+ split bias rows) — every partial
  product is exact to f32, accumulation is f32 in PSUM, the dropped zl.wl
  term is ~1e-5 absolute.  f32-grade accuracy at 1 cycle/row instead of
  native fp32's 4 cycles/row.

  Per 128-token tile:
    - PE: 16 matmuls (fp32r) -> q in PSUM [128,1024] f32
    - DVE tensor_tensor_scan (op0=max): pm = prefix-max of q -> SBUF;
      pm[:,-1] is the row max.
    - ACT: idx = accum(Sign(tmax - pm)) — counts elements before the first
      occurrence of the max (exact f32 compares, exact tie semantics);
      also copies pm[:,-1] (row max, used for the loss) to an output buffer.
    - GPSIMD indirect DMA: z_q rows gathered from weight in HBM by idx.
  loss: sum d_min = sum z^2 - sum_n max_k q  (host f64 final reduction).
"""

import numpy as np

N_TOK = 65536
K_CODES = 1024
D_DIM = 256
N_CORES = 8
TOK_PER_CORE = N_TOK // N_CORES          # 8192
TILES = TOK_PER_CORE // 128              # 64
BETA = 0.25

_cache = {}


def _install_birpatch():
    """Split multi-wait instructions in the BIR: this walrus build allows only
    one sem-wait per instruction, but Tile's kernel-tail drain carries one
    wait per semaphore lane. Extra waits move to single-wait Drain carriers."""
    if _cache.get("birpatch"):
        return
    _cache["birpatch"] = True
    import json as _json

    import concourse.bass_utils as _bu
    import concourse.bass2jax as _b2j

    def _split_multiwait(bir_bytes):
        m = _json.loads(bir_bytes)
        changed = False
        for fn in m.get("functions", []):
            for bb in fn.get("blocks", []):
                out = []
                for inst in bb.get("instructions", []):
                    si = inst.get("sync_info") or {}
                    waits = si.get("on_wait") or []
                    if len(waits) <= 1:
                        out.append(inst)
                        continue
                    changed = True
                    for i, w in enumerate(waits[:-1]):
                        out.append(
                            {
                                "name": f"{inst['name']}-w{i}",
                                "opcode": "EventSemaphore",
                                "engine": inst["engine"],
                                "ins": [],
                                "outs": [],
                                "debug": inst.get("debug", 0),
                                "sync_info": {"on_update": [], "on_wait": [w]},
                            }
                        )
                    si["on_wait"] = [waits[-1]]
                    inst["sync_info"] = si
                    out.append(inst)
                bb["instructions"] = out
        return _json.dumps(m).encode() if changed else bir_bytes

    orig = _bu.compile_bir_kernel

    def patched(bir_json, tmpdir, neff_name="file.neff"):
        if isinstance(bir_json, str):
            bir_json = bir_json.encode()
        return orig(_split_multiwait(bir_json), tmpdir, neff_name)

    _bu.compile_bir_kernel = patched
    _b2j.compile_bir_kernel = patched


def _build_bass():
    _install_birpatch()
    import concourse.bass as bass
    import concourse.tile as tile
    import concourse.mybir as mybir

    f32 = mybir.dt.float32
    r32 = mybir.dt.float32r
    nc = bass.Bass()

    zsp = nc.dram_tensor("zsp", [2 * D_DIM, TOK_PER_CORE], r32, kind="ExternalInput")
    whT = nc.dram_tensor("whT", [D_DIM, K_CODES], r32, kind="ExternalInput")
    wlT = nc.dram_tensor("wlT", [D_DIM, K_CODES], r32, kind="ExternalInput")
    nhl = nc.dram_tensor("nhl", [2, K_CODES], r32, kind="ExternalInput")
    wsrc = nc.dram_tensor("wsrc", [K_CODES, D_DIM], f32, kind="ExternalInput")

    zq = nc.dram_tensor("zq", [TOK_PER_CORE, D_DIM], f32, kind="ExternalOutput")
    idxo = nc.dram_tensor("idxo", [128, TILES], mybir.dt.int32, kind="ExternalOutput")
    vmaxo = nc.dram_tensor("vmaxo", [128, TILES], f32, kind="ExternalOutput")

    with tile.TileContext(nc) as tc:
        with (
            tc.tile_pool(name="const", bufs=1) as cpool,
            tc.tile_pool(name="zin", bufs=8) as zpool,
            tc.tile_pool(name="psum", bufs=4, space="PSUM") as qpool,
            tc.tile_pool(name="pmbuf", bufs=3) as pmpool,
            tc.tile_pool(name="junk", bufs=2) as jpool,
            tc.tile_pool(name="small", bufs=1) as spool,
            tc.tile_pool(name="gat", bufs=4) as gpool,
        ):
            wha = cpool.tile([128, K_CODES], r32, tag="wha")
            nc.sync.dma_start(wha[:], whT[0:128, :])
            whb = cpool.tile([128, K_CODES], r32, tag="whb")
            nc.scalar.dma_start(whb[:], whT[128:256, :])
            wla = cpool.tile([128, K_CODES], r32, tag="wla")
            nc.gpsimd.dma_start(wla[:], wlT[0:128, :])
            wlb = cpool.tile([128, K_CODES], r32, tag="wlb")
            nc.gpsimd.dma_start(wlb[:], wlT[128:256, :])
            nhl_t = cpool.tile([2, K_CODES], r32, tag="nhl")
            nc.gpsimd.dma_start(nhl_t[:], nhl[:])
            ones_t = cpool.tile([2, 128], f32, tag="ones")
            nc.vector.memset(ones_t[:], 1.0)
            ones = ones_t[:].bitcast(r32)
            zeros = cpool.tile([128, K_CODES], f32, tag="zeros")
            nc.vector.memset(zeros[:], 0.0)

            idxf = spool.tile([128, TILES], f32, tag="idxf")
            idxi = spool.tile([128, TILES], mybir.dt.int32, tag="idxi")
            vmaxb = spool.tile([128, TILES], f32, tag="vmaxb")

            zsrc = zsp.rearrange("(c p) t -> p c t", p=128)
            for tt in range(TILES):
                zt4 = zpool.tile([128, 512], r32, tag="zt4")
                nc.sync.dma_start(
                    zt4[:].rearrange("p (c j) -> p c j", c=4),
                    zsrc[:, :, tt * 128 : (tt + 1) * 128],
                )
                zh0 = zt4[:, 0:128]
                zh1 = zt4[:, 128:256]
                zl0 = zt4[:, 256:384]
                zl1 = zt4[:, 384:512]

                q = qpool.tile([128, K_CODES], f32, tag="q")
                L, R = slice(0, 512), slice(512, 1024)
                mm = nc.tensor.matmul
                # grouped by stationary operand (5 weight loads per tile)
                mm(q[:, L], zh0, wha[:, L], start=True, stop=False)
                mm(q[:, R], zh0, wha[:, R], start=True, stop=False)
                mm(q[:, L], zh0, wla[:, L], start=False, stop=False)
                mm(q[:, R], zh0, wla[:, R], start=False, stop=False)
                mm(q[:, L], zh1, whb[:, L], start=False, stop=False)
                mm(q[:, R], zh1, whb[:, R], start=False, stop=False)
                mm(q[:, L], zh1, wlb[:, L], start=False, stop=False)
                mm(q[:, R], zh1, wlb[:, R], start=False, stop=False)
                mm(q[:, L], zl0, wha[:, L], start=False, stop=False)
                mm(q[:, R], zl0, wha[:, R], start=False, stop=False)
                mm(q[:, L], zl1, whb[:, L], start=False, stop=False)
                mm(q[:, R], zl1, whb[:, R], start=False, stop=False)
                mm(q[:, L], ones[0:2, :], nhl_t[0:2, L], start=False, stop=True)
                mm(q[:, R], ones[0:2, :], nhl_t[0:2, R], start=False, stop=True)

                pm = pmpool.tile([128, K_CODES], f32, tag="pm")
                nc.vector.tensor_tensor_scan(
                    out=pm[:],
                    data0=q[:],
                    data1=zeros[:],
                    initial=-3.0e38,
                    op0=mybir.AluOpType.max,
                    op1=mybir.AluOpType.bypass,
                )
                # idx = sum_k Sign(tmax - pm[k]) on the scalar engine
                junk = jpool.tile([128, K_CODES], f32, tag="junk")
                nc.scalar.activation(
                    junk[:],
                    pm[:],
                    mybir.ActivationFunctionType.Sign,
                    bias=pm[:, K_CODES - 1 : K_CODES],
                    scale=-1.0,
                    accum_out=idxf[:, tt : tt + 1],
                )
                # row max for the loss
                nc.scalar.copy(vmaxb[:, tt : tt + 1], pm[:, K_CODES - 1 : K_CODES])

                # f32 -> int32 cast and z_q gather, pipelined per tile
                # (cast on gpsimd: keeps DVE free for the scans and chains
                # naturally into the gpsimd-issued indirect DMA)
                nc.gpsimd.tensor_copy(idxi[:, tt : tt + 1], idxf[:, tt : tt + 1])
                g = gpool.tile([128, D_DIM], f32, tag="g")
                nc.gpsimd.indirect_dma_start(
                    out=g[:],
                    out_offset=None,
                    in_=wsrc[:],
                    in_offset=bass.IndirectOffsetOnAxis(
                        ap=idxi[:, tt : tt + 1], axis=0
                    ),
                )
                nc.sync.dma_start(zq[tt * 128 : (tt + 1) * 128, :], g[:])

            nc.sync.dma_start(idxo[:], idxi[:])
            nc.sync.dma_start(vmaxo[:], vmaxb[:])

    return nc


def _trunc10(x):
    return (x.view(np.int32) & np.int32(~((1 << 13) - 1))).view(np.float32)


def _prep_inputs(z, weight):
    zT = np.ascontiguousarray(z.T)                         # [256, 65536]
    zhT_full = _trunc10(zT)
    zlT_full = zT - zhT_full
    zsp_full = np.concatenate([zhT_full, zlT_full], axis=0)  # [512, 65536]
    w2T = np.ascontiguousarray((2.0 * weight).T)           # [256, 1024]
    whT = _trunc10(w2T)
    wlT = np.ascontiguousarray(w2T - whT)
    negwsq = -(weight.astype(np.float64) ** 2).sum(axis=1).astype(np.float32)[None, :]
    nwh = _trunc10(negwsq)
    nhl = np.ascontiguousarray(np.concatenate([nwh, negwsq - nwh], axis=0))
    in_maps = []
    for c in range(N_CORES):
        sl = slice(c * TOK_PER_CORE, (c + 1) * TOK_PER_CORE)
        in_maps.append(
            {
                "zsp": np.ascontiguousarray(zsp_full[:, sl]),
                "whT": whT,
                "wlT": wlT,
                "nhl": nhl,
                "wsrc": weight,
            }
        )
    return in_maps


def kernel(z: np.ndarray, weight: np.ndarray):
    z = np.ascontiguousarray(z, dtype=np.float32)
    weight = np.ascontiguousarray(weight, dtype=np.float32)
    assert z.shape == (N_TOK, D_DIM) and weight.shape == (K_CODES, D_DIM)

    if "nc" not in _cache:
        _cache["nc"] = _build_bass()
    nc = _cache["nc"]

    from concourse.bass_utils import run_bass_kernel_spmd

    in_maps = _prep_inputs(z, weight)
    res = run_bass_kernel_spmd(nc, in_maps, core_ids=list(range(N_CORES)))
    return _assemble(z, weight, [res.results[c] for c in range(N_CORES)])


def _assemble(z, weight, results):
    z_q = np.concatenate([r["zq"] for r in results], axis=0)
    idx = np.concatenate(
        [r["idxo"].T.reshape(-1) for r in results], axis=0
    ).astype(np.int32)
    # loss = BETA * mean(d_min); d_min[n] = ||z_n||^2 - max_k q[n,k]
    sum_vmax = sum(float(r["vmaxo"].sum(dtype=np.float64)) for r in results)
    sum_zsq = float((z.astype(np.float64) ** 2).sum())
    loss = np.float32(BETA * (sum_zsq - sum_vmax) / (N_TOK * D_DIM))
    # straight-through estimator, matching reference arithmetic exactly
    z_q_st = z + (z_q - z)
    return (loss, z_q_st, idx)


# revision 16
# speedup vs baseline: 1.1800x; 1.1800x over previous
"""EMAVectorQuantizer forward on 8 Trainium2 NeuronCores (Bass/Tile).

Reference computation:
    d[n,k] = ||z_n||^2 + ||w_k||^2 - 2 z_n.w_k          n<65536, k<1024, D=256
    idx[n] = argmin_k d[n,k]   (first occurrence)
    z_q    = w[idx];  loss = 0.25*mean((z_q - z)^2);  z_q_st = z + (z_q - z)

Strategy (data parallel over tokens, 8 cores x 8192 tokens):
  argmin_k d = argmax_k q,  q[n,k] = 2 z_n.w_k - ||w_k||^2  (||z||^2 drops).

  Matmul precision: the PE's fast fp32 mode (float32r) rounds both operands
  to ~10 mantissa bits (TF32-like) but multiplies those exactly.  We split
  each operand at 10 mantissa bits (x = xh + xl, xh = trunc10(x)) and
  compute q = zh.wh + zh.wl + zl.wh (+ split bias rows) — every partial
  product is exact to f32, accumulation is f32 in PSUM, the dropped zl.wl
  term is ~1e-5 absolute.  f32-grade accuracy at 1 cycle/row instead of
  native fp32's 4 cycles/row.

  Per 128-token tile:
    - PE: 16 matmuls (fp32r) -> q in PSUM [128,1024] f32
    - DVE tensor_tensor_scan (op0=max): pm = prefix-max of q -> SBUF;
      pm[:,-1] is the row max.
    - ACT: idx = accum(Sign(tmax - pm)) — counts elements before the first
      occurrence of the max (exact f32 compares, exact tie semantics);
      also copies pm[:,-1] (row max, used for the loss) to an output buffer.
    - GPSIMD indirect DMA: z_q rows gathered from weight in HBM by idx.
  loss: sum d_min = sum z^2 - sum_n max_k q  (host f64 final reduction).
"""

import numpy as np

N_TOK = 65536
K_CODES = 1024
D_DIM = 256
N_CORES = 8
TOK_PER_CORE = N_TOK // N_CORES          # 8192
TILES = TOK_PER_CORE // 128              # 64
BETA = 0.25

_cache = {}


def _install_birpatch():
    """Split multi-wait instructions in the BIR: this walrus build allows only
    one sem-wait per instruction, but Tile's kernel-tail drain carries one
    wait per semaphore lane. Extra waits move to single-wait Drain carriers."""
    if _cache.get("birpatch"):
        return
    _cache["birpatch"] = True
    import json as _json

    import concourse.bass_utils as _bu
    import concourse.bass2jax as _b2j

    def _split_multiwait(bir_bytes):
        m = _json.loads(bir_bytes)
        changed = False
        for fn in m.get("functions", []):
            for bb in fn.get("blocks", []):
                out = []
                for inst in bb.get("instructions", []):
                    si = inst.get("sync_info") or {}
                    waits = si.get("on_wait") or []
                    if len(waits) <= 1:
                        out.append(inst)
                        continue
                    changed = True
                    for i, w in enumerate(waits[:-1]):
                        out.append(
                            {
                                "name": f"{inst['name']}-w{i}",
                                "opcode": "EventSemaphore",
                                "engine": inst["engine"],
                                "ins": [],
                                "outs": [],
                                "debug": inst.get("debug", 0),
                                "sync_info": {"on_update": [], "on_wait": [w]},
                            }
                        )
                    si["on_wait"] = [waits[-1]]
                    inst["sync_info"] = si
                    out.append(inst)
                bb["instructions"] = out
        return _json.dumps(m).encode() if changed else bir_bytes

    orig = _bu.compile_bir_kernel

    def patched(bir_json, tmpdir, neff_name="file.neff"):
        if isinstance(bir_json, str):
            bir_json = bir_json.encode()
        return orig(_split_multiwait(bir_json), tmpdir, neff_name)

    _bu.compile_bir_kernel = patched
    _b2j.compile_bir_kernel = patched


def _build_bass():
    _install_birpatch()
    import concourse.bass as bass
    import concourse.tile as tile
    import concourse.mybir as mybir

    f32 = mybir.dt.float32
    r32 = mybir.dt.float32r
    nc = bass.Bass()

    zsp = nc.dram_tensor("zsp", [2 * D_DIM, TOK_PER_CORE], r32, kind="ExternalInput")
    whT = nc.dram_tensor("whT", [D_DIM, K_CODES], r32, kind="ExternalInput")
    wlT = nc.dram_tensor("wlT", [D_DIM, K_CODES], r32, kind="ExternalInput")
    nhl = nc.dram_tensor("nhl", [2, K_CODES], r32, kind="ExternalInput")
    wsrc = nc.dram_tensor("wsrc", [K_CODES, D_DIM], f32, kind="ExternalInput")

    zq = nc.dram_tensor("zq", [TOK_PER_CORE, D_DIM], f32, kind="ExternalOutput")
    idxo = nc.dram_tensor("idxo", [128, TILES], mybir.dt.int32, kind="ExternalOutput")
    vmaxo = nc.dram_tensor("vmaxo", [128, TILES], f32, kind="ExternalOutput")

    with tile.TileContext(nc) as tc:
        with (
            tc.tile_pool(name="const", bufs=1) as cpool,
            tc.tile_pool(name="zin", bufs=8) as zpool,
            tc.tile_pool(name="psum", bufs=2, space="PSUM") as qpool,
            tc.tile_pool(name="pmbuf", bufs=3) as pmpool,
            tc.tile_pool(name="junk", bufs=2) as jpool,
            tc.tile_pool(name="small", bufs=1) as spool,
            tc.tile_pool(name="gat", bufs=4) as gpool,
        ):
            wha = cpool.tile([128, K_CODES], r32, tag="wha")
            nc.sync.dma_start(wha[:], whT[0:128, :])
            whb = cpool.tile([128, K_CODES], r32, tag="whb")
            nc.scalar.dma_start(whb[:], whT[128:256, :])
            wla = cpool.tile([128, K_CODES], r32, tag="wla")
            nc.gpsimd.dma_start(wla[:], wlT[0:128, :])
            wlb = cpool.tile([128, K_CODES], r32, tag="wlb")
            nc.gpsimd.dma_start(wlb[:], wlT[128:256, :])
            nhl_t = cpool.tile([2, K_CODES], r32, tag="nhl")
            nc.gpsimd.dma_start(nhl_t[:], nhl[:])
            ones_t = cpool.tile([2, 128], f32, tag="ones")
            nc.vector.memset(ones_t[:], 1.0)
            ones = ones_t[:].bitcast(r32)
            zeros = cpool.tile([128, K_CODES], f32, tag="zeros")
            nc.vector.memset(zeros[:], 0.0)

            idxf = spool.tile([128, TILES], f32, tag="idxf")
            idxi = spool.tile([128, TILES], mybir.dt.int32, tag="idxi")
            vmaxb = spool.tile([128, TILES], f32, tag="vmaxb")

            zsrc = zsp.rearrange("(c p) t -> p c t", p=128)
            for pp in range(TILES // 2):
                q2 = qpool.tile([128, 2 * K_CODES], f32, tag="q")
                zts = []
                for half in range(2):
                    tt = pp * 2 + half
                    zt4 = zpool.tile([128, 512], r32, tag="zt4")
                    nc.sync.dma_start(
                        zt4[:].rearrange("p (c j) -> p c j", c=4),
                        zsrc[:, :, tt * 128 : (tt + 1) * 128],
                    )
                    zts.append(zt4)
                for half in range(2):
                    zt4 = zts[half]
                    zh0 = zt4[:, 0:128]
                    zh1 = zt4[:, 128:256]
                    zl0 = zt4[:, 256:384]
                    zl1 = zt4[:, 384:512]
                    base = half * K_CODES
                    L = slice(base, base + 512)
                    R = slice(base + 512, base + 1024)
                    mm = nc.tensor.matmul
                    mm(q2[:, L], zh0, wha[:, 0:512], start=True, stop=False)
                    mm(q2[:, R], zh0, wha[:, 512:1024], start=True, stop=False)
                    mm(q2[:, L], zh0, wla[:, 0:512], start=False, stop=False)
                    mm(q2[:, R], zh0, wla[:, 512:1024], start=False, stop=False)
                    mm(q2[:, L], zh1, whb[:, 0:512], start=False, stop=False)
                    mm(q2[:, R], zh1, whb[:, 512:1024], start=False, stop=False)
                    mm(q2[:, L], zh1, wlb[:, 0:512], start=False, stop=False)
                    mm(q2[:, R], zh1, wlb[:, 512:1024], start=False, stop=False)
                    mm(q2[:, L], zl0, wha[:, 0:512], start=False, stop=False)
                    mm(q2[:, R], zl0, wha[:, 512:1024], start=False, stop=False)
                    mm(q2[:, L], zl1, whb[:, 0:512], start=False, stop=False)
                    mm(q2[:, R], zl1, whb[:, 512:1024], start=False, stop=False)
                    mm(q2[:, L], ones[0:2, :], nhl_t[0:2, 0:512], start=False, stop=False)
                    mm(q2[:, R], ones[0:2, :], nhl_t[0:2, 512:1024], start=False,
                       stop=(half == 1))
                for half in range(2):
                    tt = pp * 2 + half
                    base = half * K_CODES
                    pm = pmpool.tile([128, K_CODES], f32, tag="pm")
                    nc.vector.tensor_tensor_scan(
                        out=pm[:],
                        data0=q2[:, base : base + K_CODES],
                        data1=zeros[:],
                        initial=-3.0e38,
                        op0=mybir.AluOpType.max,
                        op1=mybir.AluOpType.bypass,
                    )
                    junk = jpool.tile([128, K_CODES], f32, tag="junk")
                    nc.scalar.activation(
                        junk[:],
                        pm[:],
                        mybir.ActivationFunctionType.Sign,
                        bias=pm[:, K_CODES - 1 : K_CODES],
                        scale=-1.0,
                        accum_out=idxf[:, tt : tt + 1],
                    )
                    nc.scalar.copy(vmaxb[:, tt : tt + 1], pm[:, K_CODES - 1 : K_CODES])
                    nc.gpsimd.tensor_copy(idxi[:, tt : tt + 1], idxf[:, tt : tt + 1])
                    g = gpool.tile([128, D_DIM], f32, tag="g")
                    nc.gpsimd.indirect_dma_start(
                        out=g[:],
                        out_offset=None,
                        in_=wsrc[:],
                        in_offset=bass.IndirectOffsetOnAxis(
                            ap=idxi[:, tt : tt + 1], axis=0
                        ),
                    )
                    nc.sync.dma_start(zq[tt * 128 : (tt + 1) * 128, :], g[:])

            nc.sync.dma_start(idxo[:], idxi[:])
            nc.sync.dma_start(vmaxo[:], vmaxb[:])

    return nc


def _trunc10(x):
    return (x.view(np.int32) & np.int32(~((1 << 13) - 1))).view(np.float32)


def _prep_inputs(z, weight):
    zT = np.ascontiguousarray(z.T)                         # [256, 65536]
    zhT_full = _trunc10(zT)
    zlT_full = zT - zhT_full
    zsp_full = np.concatenate([zhT_full, zlT_full], axis=0)  # [512, 65536]
    w2T = np.ascontiguousarray((2.0 * weight).T)           # [256, 1024]
    whT = _trunc10(w2T)
    wlT = np.ascontiguousarray(w2T - whT)
    negwsq = -(weight.astype(np.float64) ** 2).sum(axis=1).astype(np.float32)[None, :]
    nwh = _trunc10(negwsq)
    nhl = np.ascontiguousarray(np.concatenate([nwh, negwsq - nwh], axis=0))
    in_maps = []
    for c in range(N_CORES):
        sl = slice(c * TOK_PER_CORE, (c + 1) * TOK_PER_CORE)
        in_maps.append(
            {
                "zsp": np.ascontiguousarray(zsp_full[:, sl]),
                "whT": whT,
                "wlT": wlT,
                "nhl": nhl,
                "wsrc": weight,
            }
        )
    return in_maps


def kernel(z: np.ndarray, weight: np.ndarray):
    z = np.ascontiguousarray(z, dtype=np.float32)
    weight = np.ascontiguousarray(weight, dtype=np.float32)
    assert z.shape == (N_TOK, D_DIM) and weight.shape == (K_CODES, D_DIM)

    if "nc" not in _cache:
        _cache["nc"] = _build_bass()
    nc = _cache["nc"]

    from concourse.bass_utils import run_bass_kernel_spmd

    in_maps = _prep_inputs(z, weight)
    res = run_bass_kernel_spmd(nc, in_maps, core_ids=list(range(N_CORES)))
    return _assemble(z, weight, [res.results[c] for c in range(N_CORES)])


def _assemble(z, weight, results):
    z_q = np.concatenate([r["zq"] for r in results], axis=0)
    idx = np.concatenate(
        [r["idxo"].T.reshape(-1) for r in results], axis=0
    ).astype(np.int32)
    # loss = BETA * mean(d_min); d_min[n] = ||z_n||^2 - max_k q[n,k]
    sum_vmax = sum(float(r["vmaxo"].sum(dtype=np.float64)) for r in results)
    sum_zsq = float((z.astype(np.float64) ** 2).sum())
    loss = np.float32(BETA * (sum_zsq - sum_vmax) / (N_TOK * D_DIM))
    # straight-through estimator, matching reference arithmetic exactly
    z_q_st = z + (z_q - z)
    return (loss, z_q_st, idx)


# revision 17
# speedup vs baseline: 1.1976x; 1.0150x over previous
"""EMAVectorQuantizer forward on 8 Trainium2 NeuronCores (Bass/Tile).

Reference computation:
    d[n,k] = ||z_n||^2 + ||w_k||^2 - 2 z_n.w_k          n<65536, k<1024, D=256
    idx[n] = argmin_k d[n,k]   (first occurrence)
    z_q    = w[idx];  loss = 0.25*mean((z_q - z)^2);  z_q_st = z + (z_q - z)

Strategy (data parallel over tokens, 8 cores x 8192 tokens):
  argmin_k d = argmax_k q,  q[n,k] = 2 z_n.w_k - ||w_k||^2  (||z||^2 drops).

  Matmul precision: the PE's fast fp32 mode (float32r) rounds both operands
  to ~10 mantissa bits (TF32-like) but multiplies those exactly.  We split
  each operand at 10 mantissa bits (x = xh + xl, xh = trunc10(x)) and
  compute q = zh.wh + zh.wl + zl.wh (+ split bias rows) — every partial
  product is exact to f32, accumulation is f32 in PSUM, the dropped zl.wl
  term is ~1e-5 absolute.  f32-grade accuracy at 1 cycle/row instead of
  native fp32's 4 cycles/row.

  Per 128-token tile:
    - PE: 16 matmuls (fp32r) -> q in PSUM [128,1024] f32
    - DVE tensor_tensor_scan (op0=max): pm = prefix-max of q -> SBUF;
      pm[:,-1] is the row max.
    - ACT: idx = accum(Sign(tmax - pm)) — counts elements before the first
      occurrence of the max (exact f32 compares, exact tie semantics);
      also copies pm[:,-1] (row max, used for the loss) to an output buffer.
    - GPSIMD indirect DMA: z_q rows gathered from weight in HBM by idx.
  loss: sum d_min = sum z^2 - sum_n max_k q  (host f64 final reduction).
"""

import numpy as np

N_TOK = 65536
K_CODES = 1024
D_DIM = 256
N_CORES = 8
TOK_PER_CORE = N_TOK // N_CORES          # 8192
TILES = TOK_PER_CORE // 128              # 64
BETA = 0.25

_cache = {}


def _install_birpatch():
    """Split multi-wait instructions in the BIR: this walrus build allows only
    one sem-wait per instruction, but Tile's kernel-tail drain carries one
    wait per semaphore lane. Extra waits move to single-wait Drain carriers."""
    if _cache.get("birpatch"):
        return
    _cache["birpatch"] = True
    import json as _json

    import concourse.bass_utils as _bu
    import concourse.bass2jax as _b2j

    def _split_multiwait(bir_bytes):
        m = _json.loads(bir_bytes)
        changed = False
        for fn in m.get("functions", []):
            for bb in fn.get("blocks", []):
                out = []
                for inst in bb.get("instructions", []):
                    si = inst.get("sync_info") or {}
                    waits = si.get("on_wait") or []
                    if len(waits) <= 1:
                        out.append(inst)
                        continue
                    changed = True
                    for i, w in enumerate(waits[:-1]):
                        out.append(
                            {
                                "name": f"{inst['name']}-w{i}",
                                "opcode": "EventSemaphore",
                                "engine": inst["engine"],
                                "ins": [],
                                "outs": [],
                                "debug": inst.get("debug", 0),
                                "sync_info": {"on_update": [], "on_wait": [w]},
                            }
                        )
                    si["on_wait"] = [waits[-1]]
                    inst["sync_info"] = si
                    out.append(inst)
                bb["instructions"] = out
        return _json.dumps(m).encode() if changed else bir_bytes

    orig = _bu.compile_bir_kernel

    def patched(bir_json, tmpdir, neff_name="file.neff"):
        if isinstance(bir_json, str):
            bir_json = bir_json.encode()
        return orig(_split_multiwait(bir_json), tmpdir, neff_name)

    _bu.compile_bir_kernel = patched
    _b2j.compile_bir_kernel = patched


def _build_bass():
    _install_birpatch()
    import concourse.bass as bass
    import concourse.tile as tile
    import concourse.mybir as mybir

    f32 = mybir.dt.float32
    r32 = mybir.dt.float32r
    nc = bass.Bass()

    zsp = nc.dram_tensor("zsp", [2 * D_DIM, TOK_PER_CORE], r32, kind="ExternalInput")
    whT = nc.dram_tensor("whT", [D_DIM, K_CODES], r32, kind="ExternalInput")
    wlT = nc.dram_tensor("wlT", [D_DIM, K_CODES], r32, kind="ExternalInput")
    nhl = nc.dram_tensor("nhl", [2, K_CODES], r32, kind="ExternalInput")
    wsrc = nc.dram_tensor("wsrc", [K_CODES, D_DIM], f32, kind="ExternalInput")

    zq = nc.dram_tensor("zq", [TOK_PER_CORE, D_DIM], f32, kind="ExternalOutput")
    idxo = nc.dram_tensor("idxo", [128, TILES], mybir.dt.int32, kind="ExternalOutput")
    vmaxo = nc.dram_tensor("vmaxo", [128, TILES], f32, kind="ExternalOutput")

    with tile.TileContext(nc) as tc:
        with (
            tc.tile_pool(name="const", bufs=1) as cpool,
            tc.tile_pool(name="zin", bufs=8) as zpool,
            tc.tile_pool(name="psum", bufs=4, space="PSUM") as qpool,
            tc.tile_pool(name="pmbuf", bufs=3) as pmpool,
            tc.tile_pool(name="junk", bufs=2) as jpool,
            tc.tile_pool(name="small", bufs=1) as spool,
            tc.tile_pool(name="gat", bufs=4) as gpool,
        ):
            wha = cpool.tile([128, K_CODES], r32, tag="wha")
            nc.sync.dma_start(wha[:], whT[0:128, :])
            whb = cpool.tile([128, K_CODES], r32, tag="whb")
            nc.scalar.dma_start(whb[:], whT[128:256, :])
            wla = cpool.tile([128, K_CODES], r32, tag="wla")
            nc.gpsimd.dma_start(wla[:], wlT[0:128, :])
            wlb = cpool.tile([128, K_CODES], r32, tag="wlb")
            nc.gpsimd.dma_start(wlb[:], wlT[128:256, :])
            nhl_t = cpool.tile([2, K_CODES], r32, tag="nhl")
            nc.gpsimd.dma_start(nhl_t[:], nhl[:])
            ones_t = cpool.tile([2, 128], f32, tag="ones")
            nc.vector.memset(ones_t[:], 1.0)
            ones = ones_t[:].bitcast(r32)
            zeros = cpool.tile([128, K_CODES], f32, tag="zeros")
            nc.vector.memset(zeros[:], 0.0)

            idxf = spool.tile([128, TILES], f32, tag="idxf")
            idxi = spool.tile([128, TILES], mybir.dt.int32, tag="idxi")
            vmaxb = spool.tile([128, TILES], f32, tag="vmaxb")

            zsrc = zsp.rearrange("(c p) t -> p c t", p=128)
            for tt in range(TILES):
                zt4 = zpool.tile([128, 512], r32, tag="zt4")
                nc.sync.dma_start(
                    zt4[:].rearrange("p (c j) -> p c j", c=4),
                    zsrc[:, :, tt * 128 : (tt + 1) * 128],
                )
                zh0 = zt4[:, 0:128]
                zh1 = zt4[:, 128:256]
                zl0 = zt4[:, 256:384]
                zl1 = zt4[:, 384:512]

                q = qpool.tile([128, K_CODES], f32, tag="q")
                L, R = slice(0, 512), slice(512, 1024)
                mm = nc.tensor.matmul
                # grouped by stationary operand (5 weight loads per tile)
                mm(q[:, L], zh0, wha[:, L], start=True, stop=False)
                mm(q[:, R], zh0, wha[:, R], start=True, stop=False)
                mm(q[:, L], zh0, wla[:, L], start=False, stop=False)
                mm(q[:, R], zh0, wla[:, R], start=False, stop=False)
                mm(q[:, L], zh1, whb[:, L], start=False, stop=False)
                mm(q[:, R], zh1, whb[:, R], start=False, stop=False)
                mm(q[:, L], zh1, wlb[:, L], start=False, stop=False)
                mm(q[:, R], zh1, wlb[:, R], start=False, stop=False)
                mm(q[:, L], zl0, wha[:, L], start=False, stop=False)
                mm(q[:, R], zl0, wha[:, R], start=False, stop=False)
                mm(q[:, L], zl1, whb[:, L], start=False, stop=False)
                mm(q[:, R], zl1, whb[:, R], start=False, stop=False)
                mm(q[:, L], ones[0:2, :], nhl_t[0:2, L], start=False, stop=True)
                mm(q[:, R], ones[0:2, :], nhl_t[0:2, R], start=False, stop=True)

                pm = pmpool.tile([128, K_CODES], f32, tag="pm")
                nc.vector.tensor_tensor_scan(
                    out=pm[:],
                    data0=q[:],
                    data1=zeros[:],
                    initial=-3.0e38,
                    op0=mybir.AluOpType.max,
                    op1=mybir.AluOpType.bypass,
                )
                # idx = sum_k Sign(tmax - pm[k]) on the scalar engine
                junk = jpool.tile([128, K_CODES], f32, tag="junk")
                nc.scalar.activation(
                    junk[:],
                    pm[:],
                    mybir.ActivationFunctionType.Sign,
                    bias=pm[:, K_CODES - 1 : K_CODES],
                    scale=-1.0,
                    accum_out=idxf[:, tt : tt + 1],
                )
                # row max for the loss
                nc.scalar.copy(vmaxb[:, tt : tt + 1], pm[:, K_CODES - 1 : K_CODES])

                # f32 -> int32 cast and z_q gather, pipelined per tile
                # (cast on gpsimd: keeps DVE free for the scans and chains
                # naturally into the gpsimd-issued indirect DMA)
                nc.gpsimd.tensor_copy(idxi[:, tt : tt + 1], idxf[:, tt : tt + 1])
                g = gpool.tile([128, D_DIM], f32, tag="g")
                nc.gpsimd.indirect_dma_start(
                    out=g[:],
                    out_offset=None,
                    in_=wsrc[:],
                    in_offset=bass.IndirectOffsetOnAxis(
                        ap=idxi[:, tt : tt + 1], axis=0
                    ),
                )
                nc.sync.dma_start(zq[tt * 128 : (tt + 1) * 128, :], g[:])

            nc.sync.dma_start(idxo[:], idxi[:])
            nc.sync.dma_start(vmaxo[:], vmaxb[:])

    return nc


def _trunc10(x):
    return (x.view(np.int32) & np.int32(~((1 << 13) - 1))).view(np.float32)


def _prep_inputs(z, weight):
    zT = np.ascontiguousarray(z.T)                         # [256, 65536]
    zhT_full = _trunc10(zT)
    zlT_full = zT - zhT_full
    zsp_full = np.concatenate([zhT_full, zlT_full], axis=0)  # [512, 65536]
    w2T = np.ascontiguousarray((2.0 * weight).T)           # [256, 1024]
    whT = _trunc10(w2T)
    wlT = np.ascontiguousarray(w2T - whT)
    negwsq = -(weight.astype(np.float64) ** 2).sum(axis=1).astype(np.float32)[None, :]
    nwh = _trunc10(negwsq)
    nhl = np.ascontiguousarray(np.concatenate([nwh, negwsq - nwh], axis=0))
    in_maps = []
    for c in range(N_CORES):
        sl = slice(c * TOK_PER_CORE, (c + 1) * TOK_PER_CORE)
        in_maps.append(
            {
                "zsp": np.ascontiguousarray(zsp_full[:, sl]),
                "whT": whT,
                "wlT": wlT,
                "nhl": nhl,
                "wsrc": weight,
            }
        )
    return in_maps


def kernel(z: np.ndarray, weight: np.ndarray):
    z = np.ascontiguousarray(z, dtype=np.float32)
    weight = np.ascontiguousarray(weight, dtype=np.float32)
    assert z.shape == (N_TOK, D_DIM) and weight.shape == (K_CODES, D_DIM)

    if "nc" not in _cache:
        _cache["nc"] = _build_bass()
    nc = _cache["nc"]

    from concourse.bass_utils import run_bass_kernel_spmd

    in_maps = _prep_inputs(z, weight)
    res = run_bass_kernel_spmd(nc, in_maps, core_ids=list(range(N_CORES)))
    return _assemble(z, weight, [res.results[c] for c in range(N_CORES)])


def _assemble(z, weight, results):
    z_q = np.concatenate([r["zq"] for r in results], axis=0)
    idx = np.concatenate(
        [r["idxo"].T.reshape(-1) for r in results], axis=0
    ).astype(np.int32)
    # loss = BETA * mean(d_min); d_min[n] = ||z_n||^2 - max_k q[n,k]
    sum_vmax = sum(float(r["vmaxo"].sum(dtype=np.float64)) for r in results)
    sum_zsq = float((z.astype(np.float64) ** 2).sum())
    loss = np.float32(BETA * (sum_zsq - sum_vmax) / (N_TOK * D_DIM))
    # straight-through estimator, matching reference arithmetic exactly
    z_q_st = z + (z_q - z)
    return (loss, z_q_st, idx)
